# revision 15
# baseline (speedup 1.0000x reference)
"""Trainium2 Bass kernel for nn_DIoUAnswerSpanLoss.

Algorithm
---------
The reference builds a [B, L, L] score matrix score[b,i,j] = ls[b,i] + le[b,j]
(log-softmaxed logits), masks the lower triangle and pad positions, takes
argmax over (i, j), then computes a DIoU-style loss from the resulting integer
span positions (and the ground-truth spans).

Two exact reductions make this cheap:
  1. log_softmax subtracts a per-row constant, which never changes any
     argmax, so raw logits can be used directly for position finding.
  2. max_i<=j (msl[i] + el[j]) = cummax(msl)[j] + el[j], so the [L, L]
     matrix never needs materializing - a prefix max suffices.

Per row: msl = mask ? sl : -BIG; m = cummax(msl); col = mask ? m + el : -BIG;
j* = first argmax(col); v* = m[j*]; i* = first index with m == v*.

Distribution: the whole computation per core is ~15us of straight-line work,
while an 8-core AllReduce of even 16 bytes costs ~45us of latency on this
fabric. So instead of sharding the batch and reducing, every core receives
the full input and computes the full scalar loss independently - zero
communication - and the answer is read from core 0.

Layout: each of the 32 rows occupies 4 SBUF partitions x 512 elements
(p = 4*row + seg, global index g = 512*p + f - 65536, kept negative so a
zero from a non-matching lane never wins a reduce_min). The prefix max runs
as a segmented tensor_tensor_scan; segment boundaries are stitched with a
second scan over the PE-transposed segment ends (a -1e30 additive "reset" at
each row start keeps rows independent). Per-partition (max, first-argmax)
come from the native MAX8/FIND_INDEX8 pair. Cross-partition moves use fp32
matmuls with one-hot constants (exact: every product is x*1.0 or x*0.0);
the per-row selection then runs in a [32 rows x 4 seg] domain where the row
reductions are plain per-partition reduces and every broadcast is a
per-partition tensor_scalar operand. First-occurrence argmax semantics
(matching jnp.argmax) fall out of reduce_min over (value==max)*(negative
global index). The global-index offset cancels in every DIoU term after a
single re-basing subtract, so ground-truth positions are passed pre-offset.
"""

import numpy as np

import concourse.bass as bass
import concourse.bacc as bacc
import concourse.mybir as mybir
from concourse import tile
from concourse.bass_utils import run_bass_kernel_spmd

B, L = 32, 2048
NCORES = 8
SEG = 4                    # segments (partitions) per row
FREE = L // SEG            # 512 elements per segment
P = B * SEG                # 128 partitions
NEGBIG = -1.0e30
GOFF = float(P * FREE)     # 65536: global index offset
OFF = 4096.0               # offset used for the DIoU position space

F32 = mybir.dt.float32
U16 = mybir.dt.uint16
A = mybir.AluOpType
AX = mybir.AxisListType.X

_CACHE = {}

MASK1 = [i - 1 if i % SEG else i for i in range(32)]
MASK2 = [i - 2 if i % SEG >= 2 else i for i in range(32)]


def _tt(nc, out, a, b, op):
    nc.vector.tensor_tensor(out, a, b, op)


def _build_program():
    nc = bacc.Bacc("TRN2", target_bir_lowering=False, debug=False, num_devices=NCORES)

    sl = nc.dram_tensor("sl", [P, FREE], F32, kind="ExternalInput")
    el = nc.dram_tensor("el", [P, FREE], F32, kind="ExternalInput")
    cid = nc.dram_tensor("cid", [P, FREE], mybir.dt.int16, kind="ExternalInput")
    # constants packed into three shape-compatible bundles (fewer DMAs)
    iotag = nc.dram_tensor("iotag", [P, FREE], F32, kind="ExternalInput")
    pk1 = nc.dram_tensor("pk1", [P, SEG + B + 2], F32, kind="ExternalInput")
    pk2 = nc.dram_tensor("pk2", [B, P + 5], F32, kind="ExternalInput")
    loss = nc.dram_tensor("loss", [1], F32, kind="ExternalOutput")

    with tile.TileContext(nc) as tc:
        with (
            tc.tile_pool(name="sb", bufs=1) as sb,
            tc.tile_pool(name="ps", bufs=4, space="PSUM") as ps,
        ):
            # ---- loads (critical ones first) ----
            sl_s = sb.tile([P, FREE], F32)
            cid_s = sb.tile([P, FREE], mybir.dt.int16)
            el_s = sb.tile([P, FREE], F32)
            iotag_t = sb.tile([P, FREE], F32)
            pk1_s = sb.tile([P, SEG + B + 2], F32)
            pk2_s = sb.tile([B, P + 5], F32)
            nc.sync.dma_start(cid_s[:], cid[:])
            nc.sync.dma_start(sl_s[:], sl[:])
            nc.sync.dma_start(pk1_s[:], pk1[:])
            nc.gpsimd.dma_start(el_s[:], el[:])
            nc.gpsimd.dma_start(iotag_t[:], iotag[:])
            nc.gpsimd.dma_start(pk2_s[:], pk2[:])
            iotag_s = iotag_t[:]
            sel4_s = pk1_s[:, 0:SEG]
            ematt_s = pk1_s[:, SEG : SEG + B]
            rst1_s = pk1_s[:, SEG + B : SEG + B + 1]
            rst2_s = pk1_s[:, SEG + B + 1 : SEG + B + 2]
            emat_s = pk2_s[:, 0:P]
            ones32_s = pk2_s[:, P : P + 1]
            gt32_s = pk2_s[:, P + 1 : P + 3]
            rb32_s = pk2_s[:, P + 3 : P + 5]

            # ---- masked start logits ----
            penalty = sb.tile([P, FREE], F32)
            nc.vector.tensor_scalar(
                penalty[:], cid_s[:], 0.0, NEGBIG, A.is_equal, A.mult
            )
            msl = sb.tile([P, FREE], F32)
            _tt(nc, msl[:], sl_s[:], penalty[:], A.add)

            # ---- segmented prefix max (within each 512-elem segment) ----
            mseg = sb.tile([P, FREE], F32)
            nc.vector.tensor_tensor_scan(
                mseg[:], msl[:], msl[:], NEGBIG, A.max, A.max
            )

            # ---- stitch segments: exclusive cross-segment prefix max ----
            # partition-shuffle Hillis-Steele over the 4 segments of each row;
            # rst1/rst2 add -1e30 where a shift crossed a row boundary
            ends = mseg[:, FREE - 1 : FREE]
            sh1 = sb.tile([P, 1], F32)
            nc.vector.stream_shuffle(sh1[:], ends, MASK1)
            m1s = sb.tile([P, 1], F32)
            nc.vector.scalar_tensor_tensor(m1s[:], sh1[:], rst1_s, ends, A.add, A.max)
            sh2 = sb.tile([P, 1], F32)
            nc.vector.stream_shuffle(sh2[:], m1s[:], MASK2)
            m2s = sb.tile([P, 1], F32)
            nc.vector.scalar_tensor_tensor(m2s[:], sh2[:], rst2_s, m1s[:], A.add, A.max)
            sh3 = sb.tile([P, 1], F32)
            nc.vector.stream_shuffle(sh3[:], m2s[:], MASK1)
            exclf = sb.tile([P, 1], F32)
            nc.vector.tensor_scalar(exclf[:], sh3[:], rst1_s, None, A.add)
            m = sb.tile([P, FREE], F32)
            nc.vector.tensor_scalar(m[:], mseg[:], exclf[:], None, A.max)

            # ---- column scores: col = m + el + penalty ----
            mel = sb.tile([P, FREE], F32)
            _tt(nc, mel[:], el_s[:], penalty[:], A.add)
            col = sb.tile([P, FREE], F32)
            _tt(nc, col[:], m[:], mel[:], A.add)

            # ---- per-partition top-1 value + first index of col ----
            top8 = sb.tile([P, 8], F32)
            nc.vector.max(top8[:], col[:])
            pm = top8[:, 0:1]
            idx8 = sb.tile([P, 8], U16)
            nc.vector.max_index(idx8[:], top8[:], col[:])
            idxf = sb.tile([P, 1], F32)
            nc.vector.tensor_copy(idxf[:], idx8[:, 0:1])
            pjt = sb.tile([P, 1], F32)
            pj = pjt[:]
            nc.vector.tensor_scalar(pj, idxf[:], iotag_t[:, 0:1], None, A.add)

            # ---- per-partition m value at that index (exact gather) ----
            candv = sb.tile([P, FREE], F32)
            nc.vector.scalar_tensor_tensor(
                candv[:], iotag_s, pj, m[:], A.is_equal, A.mult
            )
            pvt = sb.tile([P, 1], F32)
            pv = pvt[:]
            nc.vector.reduce_sum(pv, candv[:], axis=AX)

            # ---- fold [128,1] vectors into [32 rows x 4 seg] ----
            rhs12 = sb.tile([P, 3 * SEG], F32)
            nc.vector.tensor_scalar(rhs12[:, 0:SEG], sel4_s, pm, None, A.mult)
            nc.vector.tensor_scalar(
                rhs12[:, SEG : 2 * SEG], sel4_s, pj, None, A.mult
            )
            nc.vector.tensor_scalar(
                rhs12[:, 2 * SEG : 3 * SEG], sel4_s, pv, None, A.mult
            )
            psT = ps.tile([B, 3 * SEG], F32, tag="ps")
            nc.tensor.matmul(psT[:], ematt_s, rhs12[:])
            sT = sb.tile([B, 3 * SEG], F32)
            nc.vector.tensor_copy(sT[:], psT[:])
            pmr = sT[:, 0:SEG]
            pjr = sT[:, SEG : 2 * SEG]
            pvr = sT[:, 2 * SEG : 3 * SEG]

            # ---- per-row selection, all per-partition now ----
            m4 = sb.tile([B, 1], F32)
            nc.vector.reduce_max(m4[:], pmr, axis=AX)
            c1 = sb.tile([B, SEG], F32)
            nc.vector.tensor_scalar(c1[:], pmr, m4[:], None, A.is_equal)
            cj = sb.tile([B, SEG], F32)
            _tt(nc, cj[:], c1[:], pjr, A.mult)
            pos32 = sb.tile([B, 2], F32)
            j4 = pos32[:, 1:2]
            nc.vector.tensor_reduce(j4, cj[:], axis=AX, op=A.min)
            c2 = sb.tile([B, SEG], F32)
            nc.vector.tensor_scalar(c2[:], cj[:], j4, None, A.is_equal)
            cv = sb.tile([B, SEG], F32)
            _tt(nc, cv[:], c2[:], pvr, A.mult)
            v4 = sb.tile([B, 1], F32)
            nc.vector.reduce_sum(v4[:], cv[:], axis=AX)

            # ---- broadcast v* back per partition, find i* ----
            psVb = ps.tile([P, 1], F32, tag="ps")
            nc.tensor.matmul(psVb[:], emat_s, v4[:])
            candi = sb.tile([P, FREE], F32)
            nc.vector.scalar_tensor_tensor(
                candi[:], m[:], psVb[:], iotag_s, A.is_equal, A.mult
            )
            pit = sb.tile([P, 1], F32)
            pi = pit[:]
            nc.vector.tensor_reduce(pi, candi[:], axis=AX, op=A.min)
            rhs4 = sb.tile([P, SEG], F32)
            nc.vector.tensor_scalar(rhs4[:], sel4_s, pi, None, A.mult)
            psI = ps.tile([B, SEG], F32, tag="ps")
            nc.tensor.matmul(psI[:], ematt_s, rhs4[:])
            i4 = pos32[:, 0:1]
            nc.vector.tensor_reduce(i4, psI[:], axis=AX, op=A.min)

            # ---- DIoU from positions, [32 x *] domain ----
            posn = sb.tile([B, 2], F32)     # i*-4096 || j*-4096 per row
            _tt(nc, posn[:], pos32[:], rb32_s, A.subtract)
            ct2 = sb.tile([B, 2], F32)      # te = ep-sp || tg = gep-gsp
            _tt(nc, ct2[:, 0:1], posn[:, 1:2], posn[:, 0:1], A.subtract)
            _tt(nc, ct2[:, 1:2], gt32_s[:, 1:2], gt32_s[:, 0:1], A.subtract)
            cdg = sb.tile([B, 2], F32)      # cd || gcd = (x+1)*0.5
            nc.vector.tensor_scalar(cdg[:], ct2[:], 1.0, 0.5, A.add, A.mult)
            mnx = sb.tile([B, 2], F32)      # min(sp,gsp) || min(ep,gep)
            _tt(nc, mnx[:], posn[:], gt32_s, A.min)
            mxx = sb.tile([B, 2], F32)      # max(sp,gsp) || max(ep,gep)
            _tt(nc, mxx[:], posn[:], gt32_s, A.max)
            dd = sb.tile([B, 2], F32)       # cd-gcd || max_end-min_start
            _tt(nc, dd[:, 0:1], cdg[:, 0:1], cdg[:, 1:2], A.subtract)
            _tt(nc, dd[:, 1:2], mxx[:, 1:2], mnx[:, 0:1], A.subtract)
            sq3 = sb.tile([B, 3], F32)      # d1^2 || d2^2 || te+tg
            _tt(nc, sq3[:, 0:2], dd[:], dd[:], A.mult)
            _tt(nc, sq3[:, 2:3], ct2[:, 0:1], ct2[:, 1:2], A.add)
            dI = sb.tile([1, 1], F32)       # I = min(ep,gep)[0] - max(sp,gsp)[0]
            _tt(nc, dI[:], mnx[0:1, 1:2], mxx[0:1, 0:1], A.subtract)
            psS = ps.tile([1, 3], F32, tag="ps")
            nc.tensor.matmul(psS[:], ones32_s, sq3[:])

            # loss = 1 - I/(s3 - 32*I) + s1/s2
            u = sb.tile([1, 1], F32)
            nc.vector.scalar_tensor_tensor(
                u[:], dI[:], -float(B), psS[0:1, 2:3], A.mult, A.add
            )
            ru = sb.tile([1, 1], F32)
            nc.vector.reciprocal(ru[:], u[:])
            iou = sb.tile([1, 1], F32)
            _tt(nc, iou[:], dI[:], ru[:], A.mult)
            r2 = sb.tile([1, 1], F32)
            nc.vector.reciprocal(r2[:], psS[0:1, 1:2])
            cl = sb.tile([1, 1], F32)
            _tt(nc, cl[:], psS[0:1, 0:1], r2[:], A.mult)
            tmp2 = sb.tile([1, 1], F32)
            nc.vector.scalar_tensor_tensor(
                tmp2[:], iou[:], -1.0, cl[:], A.mult, A.add
            )
            lossv = sb.tile([1, 1], F32)
            nc.vector.tensor_scalar(lossv[:], tmp2[:], 1.0, None, A.add)
            nc.sync.dma_start(loss[:], lossv[:])

    nc.compile()
    return nc


def _constants():
    ones1 = np.ones((1, 1), dtype=np.float32)
    ones32 = np.ones((B, 1), dtype=np.float32)
    emat = np.zeros((B, P), dtype=np.float32)
    for k in range(B):
        emat[k, k * SEG : (k + 1) * SEG] = 1.0
    ematt = np.ascontiguousarray(emat.T)
    sel4 = (np.arange(P)[:, None] % SEG == np.arange(SEG)[None, :]).astype(np.float32)
    resetv = np.zeros((1, P), dtype=np.float32)
    resetv[0, ::SEG] = NEGBIG
    id128 = np.eye(P, dtype=np.float32)
    iotag = (
        float(FREE) * np.arange(P, dtype=np.float32)[:, None]
        + np.arange(FREE, dtype=np.float32)[None, :]
        - GOFF
    ).astype(np.float32)
    # global row base in (idx - 65536) space relative to the (idx - 4096) space
    rb32 = np.repeat(
        (2048.0 * np.arange(B, dtype=np.float32) - (GOFF - OFF))[:, None], 2, axis=1
    ).astype(np.float32)
    rst1 = np.where(np.arange(P) % SEG == 0, NEGBIG, 0.0).reshape(P, 1)
    rst2 = np.where(np.arange(P) % SEG < 2, NEGBIG, 0.0).reshape(P, 1)
    pk1 = np.concatenate([sel4, ematt, rst1, rst2], axis=1).astype(np.float32)
    return {"pk1": pk1, "emat": emat, "ones32": ones32, "rb32": rb32, "iotag": iotag}


def build_in_maps(c_ids, gt_start_positions, gt_end_positions, start_logits, end_logits):
    consts = _constants()
    cidf = np.ascontiguousarray(np.asarray(c_ids), dtype=np.int16).reshape(P, FREE)
    slf = np.ascontiguousarray(np.asarray(start_logits), dtype=np.float32).reshape(P, FREE)
    elf = np.ascontiguousarray(np.asarray(end_logits), dtype=np.float32).reshape(P, FREE)
    gt32 = np.stack(
        [
            np.asarray(gt_start_positions).astype(np.float32) - np.float32(OFF),
            np.asarray(gt_end_positions).astype(np.float32) - np.float32(OFF),
        ],
        axis=1,
    ).astype(np.float32)
    pk2 = np.concatenate(
        [consts["emat"], consts["ones32"], gt32, consts["rb32"]], axis=1
    ).astype(np.float32)
    core_map = {
        "sl": slf, "el": elf, "cid": cidf,
        "pk1": consts["pk1"], "pk2": pk2, "iotag": consts["iotag"],
    }
    return [dict(core_map) for _ in range(NCORES)]


def kernel(c_ids, gt_start_positions, gt_end_positions, start_logits, end_logits):
    if "nc" not in _CACHE:
        _CACHE["nc"] = _build_program()
    nc = _CACHE["nc"]
    in_maps = build_in_maps(
        c_ids, gt_start_positions, gt_end_positions, start_logits, end_logits
    )
    res = run_bass_kernel_spmd(nc, in_maps, core_ids=list(range(NCORES)))
    return np.asarray(res.results[0]["loss"], dtype=np.float32).reshape(())


# revision 18
# speedup vs baseline: 1.1044x; 1.1044x over previous
"""Trainium2 Bass kernel for nn_DIoUAnswerSpanLoss.

Algorithm
---------
The reference builds a [B, L, L] score matrix score[b,i,j] = ls[b,i] + le[b,j]
(log-softmaxed logits), masks the lower triangle and pad positions, takes
argmax over (i, j), then computes a DIoU-style loss from the resulting integer
span positions (and the ground-truth spans).

Two exact reductions make this cheap:
  1. log_softmax subtracts a per-row constant, which never changes any
     argmax, so raw logits can be used directly for position finding.
  2. max_i<=j (msl[i] + el[j]) = cummax(msl)[j] + el[j], so the [L, L]
     matrix never needs materializing - a prefix max suffices.

Per row: msl = mask ? sl : -BIG; m = cummax(msl); col = mask ? m + el : -BIG;
j* = first argmax(col); v* = m[j*]; i* = first index with m == v*.

Distribution: the whole computation per core is ~15us of straight-line work,
while an 8-core AllReduce of even 16 bytes costs ~45us of latency on this
fabric. So instead of sharding the batch and reducing, every core receives
the full input and computes the full scalar loss independently - zero
communication - and the answer is read from core 0.

Layout: each of the 32 rows occupies 4 SBUF partitions x 512 elements
(p = 4*row + seg, global index g = 512*p + f - 65536, kept negative so a
zero from a non-matching lane never wins a reduce_min). The prefix max runs
as a segmented tensor_tensor_scan; segment boundaries are stitched by a
partition-shuffle Hillis-Steele pass (stream_shuffle shifts within each row's
4 partitions; a -1e30 additive mask kills shifts that crossed a row start).
Per-partition (max, first-argmax) come from the native MAX8/FIND_INDEX8 pair.
Cross-partition folds use fp32 matmuls with one-hot constants (exact: every
product is x*1.0 or x*0.0);
the per-row selection then runs in a [32 rows x 4 seg] domain where the row
reductions are plain per-partition reduces and every broadcast is a
per-partition tensor_scalar operand. First-occurrence argmax semantics
(matching jnp.argmax) fall out of reduce_min over (value==max)*(negative
global index). The global-index offset cancels in every DIoU term after a
single re-basing subtract, so ground-truth positions are passed pre-offset.
"""

import numpy as np

import concourse.bass as bass
import concourse.bacc as bacc
import concourse.mybir as mybir
from concourse import tile
from concourse.bass_utils import run_bass_kernel_spmd

B, L = 32, 2048
NCORES = 8
SEG = 4                    # segments (partitions) per row
FREE = L // SEG            # 512 elements per segment
P = B * SEG                # 128 partitions
NEGBIG = -1.0e30
GOFF = float(P * FREE)     # 65536: global index offset
OFF = 4096.0               # offset used for the DIoU position space

F32 = mybir.dt.float32
U16 = mybir.dt.uint16
A = mybir.AluOpType
AX = mybir.AxisListType.X

_CACHE = {}

MASK1 = [i - 1 if i % SEG else i for i in range(32)]
MASK2 = [i - 2 if i % SEG >= 2 else i for i in range(32)]


def _tt(nc, out, a, b, op):
    nc.vector.tensor_tensor(out, a, b, op)


def _build_program():
    nc = bacc.Bacc("TRN2", target_bir_lowering=False, debug=False, num_devices=NCORES)

    sl = nc.dram_tensor("sl", [P, FREE], F32, kind="ExternalInput")
    el = nc.dram_tensor("el", [P, FREE], F32, kind="ExternalInput")
    cid = nc.dram_tensor("cid", [P, FREE], mybir.dt.int16, kind="ExternalInput")
    # constants packed into three shape-compatible bundles (fewer DMAs)
    iotag = nc.dram_tensor("iotag", [P, FREE], F32, kind="ExternalInput")
    pk1 = nc.dram_tensor("pk1", [P, SEG + B + 2], F32, kind="ExternalInput")
    pk2 = nc.dram_tensor("pk2", [B, P + 5], F32, kind="ExternalInput")
    loss = nc.dram_tensor("loss", [1], F32, kind="ExternalOutput")

    with tile.TileContext(nc) as tc:
        with (
            tc.tile_pool(name="sb", bufs=1) as sb,
            tc.tile_pool(name="ps", bufs=4, space="PSUM") as ps,
        ):
            # ---- loads (critical ones first) ----
            sl_s = sb.tile([P, FREE], F32)
            cid_s = sb.tile([P, FREE], mybir.dt.int16)
            el_s = sb.tile([P, FREE], F32)
            iotag_t = sb.tile([P, FREE], F32)
            pk1_s = sb.tile([P, SEG + B + 2], F32)
            pk2_s = sb.tile([B, P + 5], F32)
            nc.sync.dma_start(sl_s[:], sl[:])
            nc.sync.dma_start(cid_s[:], cid[:])
            nc.sync.dma_start(pk1_s[:], pk1[:])
            nc.gpsimd.dma_start(el_s[:], el[:])
            nc.gpsimd.dma_start(iotag_t[:], iotag[:])
            nc.gpsimd.dma_start(pk2_s[:], pk2[:])
            iotag_s = iotag_t[:]
            sel4_s = pk1_s[:, 0:SEG]
            ematt_s = pk1_s[:, SEG : SEG + B]
            rst1_s = pk1_s[:, SEG + B : SEG + B + 1]
            rst2_s = pk1_s[:, SEG + B + 1 : SEG + B + 2]
            emat_s = pk2_s[:, 0:P]
            ones32_s = pk2_s[:, P : P + 1]
            gt32_s = pk2_s[:, P + 1 : P + 3]
            rb32_s = pk2_s[:, P + 3 : P + 5]

            # ---- masked start logits ----
            penalty = sb.tile([P, FREE], F32)
            nc.vector.tensor_scalar(
                penalty[:], cid_s[:], 0.0, NEGBIG, A.is_equal, A.mult
            )
            msl = sb.tile([P, FREE], F32)
            _tt(nc, msl[:], sl_s[:], penalty[:], A.add)

            # ---- segmented prefix max (within each 512-elem segment) ----
            mseg = sb.tile([P, FREE], F32)
            nc.vector.tensor_tensor_scan(
                mseg[:], msl[:], msl[:], NEGBIG, A.max, A.max
            )

            # ---- stitch segments: exclusive cross-segment prefix max ----
            # partition-shuffle Hillis-Steele over the 4 segments of each row;
            # rst1/rst2 add -1e30 where a shift crossed a row boundary
            ends = mseg[:, FREE - 1 : FREE]
            sh1 = sb.tile([P, 1], F32)
            nc.vector.stream_shuffle(sh1[:], ends, MASK1)
            m1s = sb.tile([P, 1], F32)
            nc.vector.scalar_tensor_tensor(m1s[:], sh1[:], rst1_s, ends, A.add, A.max)
            sh2 = sb.tile([P, 1], F32)
            nc.vector.stream_shuffle(sh2[:], m1s[:], MASK2)
            m2s = sb.tile([P, 1], F32)
            nc.vector.scalar_tensor_tensor(m2s[:], sh2[:], rst2_s, m1s[:], A.add, A.max)
            sh3 = sb.tile([P, 1], F32)
            nc.vector.stream_shuffle(sh3[:], m2s[:], MASK1)
            exclf = sb.tile([P, 1], F32)
            nc.vector.tensor_scalar(exclf[:], sh3[:], rst1_s, None, A.add)

            # ---- column scores: col = max(mseg, excl) + (el + penalty) ----
            mel = sb.tile([P, FREE], F32)
            _tt(nc, mel[:], el_s[:], penalty[:], A.add)
            col = sb.tile([P, FREE], F32)
            nc.vector.scalar_tensor_tensor(
                col[:], mseg[:], exclf[:], mel[:], A.max, A.add
            )

            # ---- per-partition top-1 value + first index of col ----
            top8 = sb.tile([P, 8], F32)
            nc.vector.max(top8[:], col[:])
            pm = top8[:, 0:1]
            idx8 = sb.tile([P, 8], U16)
            nc.vector.max_index(idx8[:], top8[:], col[:])
            idxf = sb.tile([P, 1], F32)
            nc.vector.tensor_copy(idxf[:], idx8[:, 0:1])
            pjt = sb.tile([P, 1], F32)
            pj = pjt[:]
            nc.vector.tensor_scalar(pj, idxf[:], iotag_t[:, 0:1], None, A.add)

            # ---- per-partition m value at that index (exact gather) ----
            # gather mseg at the argmax position, then fold in the carried
            # prefix: m[pj] = max(mseg[pj], excl[partition])
            candv = sb.tile([P, FREE], F32)
            nc.vector.scalar_tensor_tensor(
                candv[:], iotag_s, pj, mseg[:], A.is_equal, A.mult
            )
            pvseg = sb.tile([P, 1], F32)
            nc.vector.reduce_sum(pvseg[:], candv[:], axis=AX)
            pvt = sb.tile([P, 1], F32)
            pv = pvt[:]
            nc.vector.tensor_scalar(pv, pvseg[:], exclf[:], None, A.max)

            # ---- fold [128,1] vectors into [32 rows x 4 seg] ----
            rhs12 = sb.tile([P, 3 * SEG], F32)
            nc.vector.tensor_scalar(rhs12[:, 0:SEG], sel4_s, pm, None, A.mult)
            nc.vector.tensor_scalar(
                rhs12[:, SEG : 2 * SEG], sel4_s, pj, None, A.mult
            )
            nc.vector.tensor_scalar(
                rhs12[:, 2 * SEG : 3 * SEG], sel4_s, pv, None, A.mult
            )
            psT = ps.tile([B, 3 * SEG], F32, tag="ps")
            nc.tensor.matmul(psT[:], ematt_s, rhs12[:])
            sT = sb.tile([B, 3 * SEG], F32)
            nc.vector.tensor_copy(sT[:], psT[:])
            pmr = sT[:, 0:SEG]
            pjr = sT[:, SEG : 2 * SEG]
            pvr = sT[:, 2 * SEG : 3 * SEG]

            # ---- per-row selection, all per-partition now ----
            m4 = sb.tile([B, 1], F32)
            nc.vector.reduce_max(m4[:], pmr, axis=AX)
            c1 = sb.tile([B, SEG], F32)
            nc.vector.tensor_scalar(c1[:], pmr, m4[:], None, A.is_equal)
            cj = sb.tile([B, SEG], F32)
            _tt(nc, cj[:], c1[:], pjr, A.mult)
            pos32 = sb.tile([B, 2], F32)
            j4 = pos32[:, 1:2]
            nc.vector.tensor_reduce(j4, cj[:], axis=AX, op=A.min)
            c2 = sb.tile([B, SEG], F32)
            nc.vector.tensor_scalar(c2[:], cj[:], j4, None, A.is_equal)
            cv = sb.tile([B, SEG], F32)
            _tt(nc, cv[:], c2[:], pvr, A.mult)
            v4 = sb.tile([B, 1], F32)
            nc.vector.reduce_sum(v4[:], cv[:], axis=AX)

            # ---- broadcast v* back per partition, find i* ----
            psVb = ps.tile([P, 1], F32, tag="ps")
            nc.tensor.matmul(psVb[:], emat_s, v4[:])
            candi = sb.tile([P, FREE], F32)
            nc.vector.scalar_tensor_tensor(
                candi[:], msl[:], psVb[:], iotag_s, A.is_equal, A.mult
            )
            pit = sb.tile([P, 1], F32)
            pi = pit[:]
            nc.vector.tensor_reduce(pi, candi[:], axis=AX, op=A.min)
            rhs4 = sb.tile([P, SEG], F32)
            nc.vector.tensor_scalar(rhs4[:], sel4_s, pi, None, A.mult)
            psI = ps.tile([B, SEG], F32, tag="ps")
            nc.tensor.matmul(psI[:], ematt_s, rhs4[:])
            i4 = pos32[:, 0:1]
            nc.vector.tensor_reduce(i4, psI[:], axis=AX, op=A.min)

            # ---- DIoU from positions, [32 x *] domain ----
            posn = sb.tile([B, 2], F32)     # i*-4096 || j*-4096 per row
            _tt(nc, posn[:], pos32[:], rb32_s, A.subtract)
            ct2 = sb.tile([B, 2], F32)      # te = ep-sp || tg = gep-gsp
            _tt(nc, ct2[:, 0:1], posn[:, 1:2], posn[:, 0:1], A.subtract)
            _tt(nc, ct2[:, 1:2], gt32_s[:, 1:2], gt32_s[:, 0:1], A.subtract)
            cdg = sb.tile([B, 2], F32)      # cd || gcd = (x+1)*0.5
            nc.vector.tensor_scalar(cdg[:], ct2[:], 1.0, 0.5, A.add, A.mult)
            mnx = sb.tile([B, 2], F32)      # min(sp,gsp) || min(ep,gep)
            _tt(nc, mnx[:], posn[:], gt32_s, A.min)
            mxx = sb.tile([B, 2], F32)      # max(sp,gsp) || max(ep,gep)
            _tt(nc, mxx[:], posn[:], gt32_s, A.max)
            dd = sb.tile([B, 2], F32)       # cd-gcd || max_end-min_start
            _tt(nc, dd[:, 0:1], cdg[:, 0:1], cdg[:, 1:2], A.subtract)
            _tt(nc, dd[:, 1:2], mxx[:, 1:2], mnx[:, 0:1], A.subtract)
            sq3 = sb.tile([B, 3], F32)      # d1^2 || d2^2 || te+tg
            _tt(nc, sq3[:, 0:2], dd[:], dd[:], A.mult)
            _tt(nc, sq3[:, 2:3], ct2[:, 0:1], ct2[:, 1:2], A.add)
            dI = sb.tile([1, 1], F32)       # I = min(ep,gep)[0] - max(sp,gsp)[0]
            _tt(nc, dI[:], mnx[0:1, 1:2], mxx[0:1, 0:1], A.subtract)
            psS = ps.tile([1, 3], F32, tag="ps")
            nc.tensor.matmul(psS[:], ones32_s, sq3[:])

            # loss = 1 - I/(s3 - 32*I) + s1/s2
            u = sb.tile([1, 1], F32)
            nc.vector.scalar_tensor_tensor(
                u[:], dI[:], -float(B), psS[0:1, 2:3], A.mult, A.add
            )
            ru = sb.tile([1, 1], F32)
            nc.vector.reciprocal(ru[:], u[:])
            iou = sb.tile([1, 1], F32)
            _tt(nc, iou[:], dI[:], ru[:], A.mult)
            r2 = sb.tile([1, 1], F32)
            nc.vector.reciprocal(r2[:], psS[0:1, 1:2])
            cl = sb.tile([1, 1], F32)
            _tt(nc, cl[:], psS[0:1, 0:1], r2[:], A.mult)
            tmp2 = sb.tile([1, 1], F32)
            nc.vector.scalar_tensor_tensor(
                tmp2[:], iou[:], -1.0, cl[:], A.mult, A.add
            )
            lossv = sb.tile([1, 1], F32)
            nc.vector.tensor_scalar(lossv[:], tmp2[:], 1.0, None, A.add)
            nc.sync.dma_start(loss[:], lossv[:])

    for bb in nc.main_func.blocks:
        dead = [
            ins
            for ins in bb.instructions
            if type(ins).__name__ == "InstMemset" and "@const-" in ins.concise()
        ]
        for ins in dead:
            try:
                bb.instructions.remove(ins)
            except (ValueError, AttributeError):
                pass
    nc.compile()
    return nc


def _constants():
    ones1 = np.ones((1, 1), dtype=np.float32)
    ones32 = np.ones((B, 1), dtype=np.float32)
    emat = np.zeros((B, P), dtype=np.float32)
    for k in range(B):
        emat[k, k * SEG : (k + 1) * SEG] = 1.0
    ematt = np.ascontiguousarray(emat.T)
    sel4 = (np.arange(P)[:, None] % SEG == np.arange(SEG)[None, :]).astype(np.float32)
    resetv = np.zeros((1, P), dtype=np.float32)
    resetv[0, ::SEG] = NEGBIG
    id128 = np.eye(P, dtype=np.float32)
    iotag = (
        float(FREE) * np.arange(P, dtype=np.float32)[:, None]
        + np.arange(FREE, dtype=np.float32)[None, :]
        - GOFF
    ).astype(np.float32)
    # global row base in (idx - 65536) space relative to the (idx - 4096) space
    rb32 = np.repeat(
        (2048.0 * np.arange(B, dtype=np.float32) - (GOFF - OFF))[:, None], 2, axis=1
    ).astype(np.float32)
    rst1 = np.where(np.arange(P) % SEG == 0, NEGBIG, 0.0).reshape(P, 1)
    rst2 = np.where(np.arange(P) % SEG < 2, NEGBIG, 0.0).reshape(P, 1)
    pk1 = np.concatenate([sel4, ematt, rst1, rst2], axis=1).astype(np.float32)
    return {"pk1": pk1, "emat": emat, "ones32": ones32, "rb32": rb32, "iotag": iotag}


def build_in_maps(c_ids, gt_start_positions, gt_end_positions, start_logits, end_logits):
    consts = _constants()
    cidf = np.ascontiguousarray(np.asarray(c_ids), dtype=np.int16).reshape(P, FREE)
    slf = np.ascontiguousarray(np.asarray(start_logits), dtype=np.float32).reshape(P, FREE)
    elf = np.ascontiguousarray(np.asarray(end_logits), dtype=np.float32).reshape(P, FREE)
    gt32 = np.stack(
        [
            np.asarray(gt_start_positions).astype(np.float32) - np.float32(OFF),
            np.asarray(gt_end_positions).astype(np.float32) - np.float32(OFF),
        ],
        axis=1,
    ).astype(np.float32)
    pk2 = np.concatenate(
        [consts["emat"], consts["ones32"], gt32, consts["rb32"]], axis=1
    ).astype(np.float32)
    core_map = {
        "sl": slf, "el": elf, "cid": cidf,
        "pk1": consts["pk1"], "pk2": pk2, "iotag": consts["iotag"],
    }
    return [dict(core_map) for _ in range(NCORES)]


def kernel(c_ids, gt_start_positions, gt_end_positions, start_logits, end_logits):
    if "nc" not in _CACHE:
        _CACHE["nc"] = _build_program()
    nc = _CACHE["nc"]
    in_maps = build_in_maps(
        c_ids, gt_start_positions, gt_end_positions, start_logits, end_logits
    )
    res = run_bass_kernel_spmd(nc, in_maps, core_ids=list(range(NCORES)))
    return np.asarray(res.results[0]["loss"], dtype=np.float32).reshape(())


# revision 23
# speedup vs baseline: 1.1396x; 1.0318x over previous
"""Trainium2 Bass kernel for nn_DIoUAnswerSpanLoss.

Algorithm
---------
The reference builds a [B, L, L] score matrix score[b,i,j] = ls[b,i] + le[b,j]
(log-softmaxed logits), masks the lower triangle and pad positions, takes
argmax over (i, j), then computes a DIoU-style loss from the resulting integer
span positions (and the ground-truth spans).

Two exact reductions make this cheap:
  1. log_softmax subtracts a per-row constant, which never changes any
     argmax, so raw logits can be used directly for position finding.
  2. max_i<=j (msl[i] + el[j]) = cummax(msl)[j] + el[j], so the [L, L]
     matrix never needs materializing - a prefix max suffices.

Per row: msl = mask ? sl : -BIG; m = cummax(msl); col = mask ? m + el : -BIG;
j* = first argmax(col); v* = m[j*]; i* = first index with m == v*.

Distribution: the whole computation per core is ~15us of straight-line work,
while an 8-core AllReduce of even 16 bytes costs ~45us of latency on this
fabric. So instead of sharding the batch and reducing, every core receives
the full input and computes the full scalar loss independently - zero
communication - and the answer is read from core 0.

Layout: each of the 32 rows occupies 4 SBUF partitions x 512 elements
(p = 4*row + seg, global index g = 512*p + f - 65536, kept negative so a
zero from a non-matching lane never wins a reduce_min). The prefix max runs
as a segmented tensor_tensor_scan; segment boundaries are stitched by a
partition-shuffle Hillis-Steele pass (stream_shuffle shifts within each row's
4 partitions; a -1e30 additive mask kills shifts that crossed a row start).
Per-partition (max, first-argmax) come from the native MAX8/FIND_INDEX8 pair.
Cross-partition folds use fp32 matmuls with one-hot constants (exact: every
product is x*1.0 or x*0.0);
the per-row selection then runs in a [32 rows x 4 seg] domain where the row
reductions are plain per-partition reduces and every broadcast is a
per-partition tensor_scalar operand. First-occurrence argmax semantics
(matching jnp.argmax) fall out of reduce_min over (value==max)*(negative
global index). The global-index offset cancels in every DIoU term after a
single re-basing subtract, so ground-truth positions are passed pre-offset.
"""

import numpy as np

import concourse.bass as bass
import concourse.bacc as bacc
import concourse.mybir as mybir
from concourse import tile
from concourse.tile import add_dep_helper
from concourse.bass_utils import run_bass_kernel_spmd

B, L = 32, 2048
NCORES = 8
SEG = 4                    # segments (partitions) per row
FREE = L // SEG            # 512 elements per segment
P = B * SEG                # 128 partitions
NEGBIG = -1.0e30
GOFF = float(P * FREE)     # 65536: global index offset
OFF = 4096.0               # offset used for the DIoU position space

F32 = mybir.dt.float32
U16 = mybir.dt.uint16
A = mybir.AluOpType
AX = mybir.AxisListType.X

_CACHE = {}

MASK1 = [i - 1 if i % SEG else i for i in range(32)]
MASK2 = [i - 2 if i % SEG >= 2 else i for i in range(32)]


def _tt(nc, out, a, b, op):
    nc.vector.tensor_tensor(out, a, b, op)


def _build_program():
    nc = bacc.Bacc("TRN2", target_bir_lowering=False, debug=False, num_devices=NCORES)

    sl = nc.dram_tensor("sl", [P, FREE], F32, kind="ExternalInput")
    el = nc.dram_tensor("el", [P, FREE], F32, kind="ExternalInput")
    cid = nc.dram_tensor("cid", [P, FREE], mybir.dt.int16, kind="ExternalInput")
    # constants packed into three shape-compatible bundles (fewer DMAs)
    iotag = nc.dram_tensor("iotag", [P, FREE], F32, kind="ExternalInput")
    pk1 = nc.dram_tensor("pk1", [P, SEG + B + 2], F32, kind="ExternalInput")
    pk2 = nc.dram_tensor("pk2", [B, P + 5], F32, kind="ExternalInput")
    loss = nc.dram_tensor("loss", [1], F32, kind="ExternalOutput")

    with tile.TileContext(nc) as tc:
        with (
            tc.tile_pool(name="sb", bufs=1) as sb,
            tc.tile_pool(name="ps", bufs=4, space="PSUM") as ps,
        ):
            # ---- loads (critical ones first) ----
            sl_s = sb.tile([P, FREE], F32)
            cid_s = sb.tile([P, FREE], mybir.dt.int16)
            el_s = sb.tile([P, FREE], F32)
            iotag_t = sb.tile([P, FREE], F32)
            pk1_s = sb.tile([P, SEG + B + 2], F32)
            pk2_s = sb.tile([B, P + 5], F32)
            nc.sync.dma_start(sl_s[:], sl[:])
            nc.sync.dma_start(cid_s[:], cid[:])
            nc.sync.dma_start(pk1_s[:], pk1[:])
            nc.gpsimd.dma_start(pk2_s[:], pk2[:])
            nc.gpsimd.dma_start(el_s[:], el[:])
            nc.gpsimd.dma_start(iotag_t[:], iotag[:])
            iotag_s = iotag_t[:]
            sel4_s = pk1_s[:, 0:SEG]
            ematt_s = pk1_s[:, SEG : SEG + B]
            rst1_s = pk1_s[:, SEG + B : SEG + B + 1]
            rst2_s = pk1_s[:, SEG + B + 1 : SEG + B + 2]
            emat_s = pk2_s[:, 0:P]
            ones32_s = pk2_s[:, P : P + 1]
            gt32_s = pk2_s[:, P + 1 : P + 3]
            rb32_s = pk2_s[:, P + 3 : P + 5]

            # ---- masked start logits ----
            penalty = sb.tile([P, FREE], F32)
            nc.vector.tensor_scalar(
                penalty[:], cid_s[:], 0.0, NEGBIG, A.is_equal, A.mult
            )
            msl = sb.tile([P, FREE], F32)
            _tt(nc, msl[:], sl_s[:], penalty[:], A.add)

            # ---- segmented prefix max (within each 512-elem segment) ----
            mseg = sb.tile([P, FREE], F32)
            scan_i = nc.vector.tensor_tensor_scan(
                mseg[:], msl[:], msl[:], NEGBIG, A.max, A.max
            )

            # ---- stitch segments: exclusive cross-segment prefix max ----
            # partition-shuffle Hillis-Steele over the 4 segments of each row;
            # rst1/rst2 add -1e30 where a shift crossed a row boundary
            ends = mseg[:, FREE - 1 : FREE]
            sh1 = sb.tile([P, 1], F32)
            nc.vector.stream_shuffle(sh1[:], ends, MASK1)
            m1s = sb.tile([P, 1], F32)
            nc.vector.scalar_tensor_tensor(m1s[:], sh1[:], rst1_s, ends, A.add, A.max)
            sh2 = sb.tile([P, 1], F32)
            nc.vector.stream_shuffle(sh2[:], m1s[:], MASK2)
            m2s = sb.tile([P, 1], F32)
            nc.vector.scalar_tensor_tensor(m2s[:], sh2[:], rst2_s, m1s[:], A.add, A.max)
            sh3 = sb.tile([P, 1], F32)
            nc.vector.stream_shuffle(sh3[:], m2s[:], MASK1)
            exclf = sb.tile([P, 1], F32)
            nc.vector.tensor_scalar(exclf[:], sh3[:], rst1_s, None, A.add)

            # ---- column scores: col = max(mseg, excl) + (el + penalty) ----
            mel = sb.tile([P, FREE], F32)
            mel_i = nc.vector.tensor_tensor(mel[:], el_s[:], penalty[:], A.add)
            # keep the in-order DVE stream from head-of-line blocking on the
            # (later-arriving) el DMA: schedule mel after the scan
            add_dep_helper(mel_i.ins, scan_i.ins, sync=False,
                           reason="mel after scan: avoid DVE HOL block on el DMA")
            col = sb.tile([P, FREE], F32)
            nc.vector.scalar_tensor_tensor(
                col[:], mseg[:], exclf[:], mel[:], A.max, A.add
            )

            # ---- per-partition top-1 value + first index of col ----
            top8 = sb.tile([P, 8], F32)
            nc.vector.max(top8[:], col[:])
            pm = top8[:, 0:1]
            idx8 = sb.tile([P, 8], U16)
            nc.vector.max_index(idx8[:], top8[:], col[:])
            idxf = sb.tile([P, 1], F32)
            nc.vector.tensor_copy(idxf[:], idx8[:, 0:1])
            pjt = sb.tile([P, 1], F32)
            pj = pjt[:]
            nc.vector.tensor_scalar(pj, idxf[:], iotag_t[:, 0:1], None, A.add)

            # ---- per-partition m value at that index (exact gather) ----
            # gather mseg at the argmax position, then fold in the carried
            # prefix: m[pj] = max(mseg[pj], excl[partition])
            candv = sb.tile([P, FREE], F32)
            nc.vector.scalar_tensor_tensor(
                candv[:], iotag_s, pj, mseg[:], A.is_equal, A.mult
            )
            pvseg = sb.tile([P, 1], F32)
            nc.vector.reduce_sum(pvseg[:], candv[:], axis=AX)
            pvt = sb.tile([P, 1], F32)
            pv = pvt[:]
            nc.vector.tensor_scalar(pv, pvseg[:], exclf[:], None, A.max)

            # ---- fold [128,1] vectors into [32 rows x 4 seg] ----
            rhs12 = sb.tile([P, 3 * SEG], F32)
            nc.vector.tensor_scalar(rhs12[:, 0:SEG], sel4_s, pm, None, A.mult)
            nc.vector.tensor_scalar(
                rhs12[:, SEG : 2 * SEG], sel4_s, pj, None, A.mult
            )
            nc.vector.tensor_scalar(
                rhs12[:, 2 * SEG : 3 * SEG], sel4_s, pv, None, A.mult
            )
            psT = ps.tile([B, 3 * SEG], F32, tag="ps")
            nc.tensor.matmul(psT[:], ematt_s, rhs12[:])
            sT = sb.tile([B, 3 * SEG], F32)
            nc.vector.tensor_copy(sT[:], psT[:])
            pmr = sT[:, 0:SEG]
            pjr = sT[:, SEG : 2 * SEG]
            pvr = sT[:, 2 * SEG : 3 * SEG]

            # ---- per-row selection, all per-partition now ----
            m4 = sb.tile([B, 1], F32)
            nc.vector.reduce_max(m4[:], pmr, axis=AX)
            c1 = sb.tile([B, SEG], F32)
            nc.vector.tensor_scalar(c1[:], pmr, m4[:], None, A.is_equal)
            cj = sb.tile([B, SEG], F32)
            _tt(nc, cj[:], c1[:], pjr, A.mult)
            pos32 = sb.tile([B, 2], F32)
            j4 = pos32[:, 1:2]
            nc.vector.tensor_reduce(j4, cj[:], axis=AX, op=A.min)
            c2 = sb.tile([B, SEG], F32)
            nc.vector.tensor_scalar(c2[:], cj[:], j4, None, A.is_equal)
            cv = sb.tile([B, SEG], F32)
            _tt(nc, cv[:], c2[:], pvr, A.mult)
            v4 = sb.tile([B, 1], F32)
            nc.vector.reduce_sum(v4[:], cv[:], axis=AX)

            # ---- broadcast v* back per partition, find i* ----
            psVb = ps.tile([P, 1], F32, tag="ps")
            nc.tensor.matmul(psVb[:], emat_s, v4[:])
            candi = sb.tile([P, FREE], F32)
            nc.vector.scalar_tensor_tensor(
                candi[:], msl[:], psVb[:], iotag_s, A.is_equal, A.mult
            )
            pit = sb.tile([P, 1], F32)
            pi = pit[:]
            nc.vector.tensor_reduce(pi, candi[:], axis=AX, op=A.min)
            rhs4 = sb.tile([P, SEG], F32)
            nc.vector.tensor_scalar(rhs4[:], sel4_s, pi, None, A.mult)
            psI = ps.tile([B, SEG], F32, tag="ps")
            nc.tensor.matmul(psI[:], ematt_s, rhs4[:])
            i4 = pos32[:, 0:1]
            nc.vector.tensor_reduce(i4, psI[:], axis=AX, op=A.min)

            # ---- DIoU from positions, [32 x *] domain ----
            posn = sb.tile([B, 2], F32)     # i*-4096 || j*-4096 per row
            _tt(nc, posn[:], pos32[:], rb32_s, A.subtract)
            ct2 = sb.tile([B, 2], F32)      # te = ep-sp || tg = gep-gsp
            _tt(nc, ct2[:, 0:1], posn[:, 1:2], posn[:, 0:1], A.subtract)
            tgs_i = nc.vector.tensor_tensor(
                ct2[:, 1:2], gt32_s[:, 1:2], gt32_s[:, 0:1], A.subtract
            )
            add_dep_helper(tgs_i.ins, scan_i.ins, sync=False,
                           reason="t_gs after scan: avoid DVE HOL block on pk2 DMA")
            cdg = sb.tile([B, 2], F32)      # cd || gcd = (x+1)*0.5
            nc.vector.tensor_scalar(cdg[:], ct2[:], 1.0, 0.5, A.add, A.mult)
            mnx = sb.tile([B, 2], F32)      # min(sp,gsp) || min(ep,gep)
            _tt(nc, mnx[:], posn[:], gt32_s, A.min)
            mxx = sb.tile([B, 2], F32)      # max(sp,gsp) || max(ep,gep)
            _tt(nc, mxx[:], posn[:], gt32_s, A.max)
            dd = sb.tile([B, 2], F32)       # cd-gcd || max_end-min_start
            _tt(nc, dd[:, 0:1], cdg[:, 0:1], cdg[:, 1:2], A.subtract)
            _tt(nc, dd[:, 1:2], mxx[:, 1:2], mnx[:, 0:1], A.subtract)
            sq3 = sb.tile([B, 3], F32)      # d1^2 || d2^2 || te+tg
            _tt(nc, sq3[:, 0:2], dd[:], dd[:], A.mult)
            _tt(nc, sq3[:, 2:3], ct2[:, 0:1], ct2[:, 1:2], A.add)
            dI = sb.tile([1, 1], F32)       # I = min(ep,gep)[0] - max(sp,gsp)[0]
            _tt(nc, dI[:], mnx[0:1, 1:2], mxx[0:1, 0:1], A.subtract)
            psS = ps.tile([1, 3], F32, tag="ps")
            nc.tensor.matmul(psS[:], ones32_s, sq3[:])

            # loss = 1 - I/(s3 - 32*I) + s1/s2
            u = sb.tile([1, 1], F32)
            nc.vector.scalar_tensor_tensor(
                u[:], dI[:], -float(B), psS[0:1, 2:3], A.mult, A.add
            )
            ru = sb.tile([1, 1], F32)
            nc.vector.reciprocal(ru[:], u[:])
            iou = sb.tile([1, 1], F32)
            _tt(nc, iou[:], dI[:], ru[:], A.mult)
            r2 = sb.tile([1, 1], F32)
            nc.vector.reciprocal(r2[:], psS[0:1, 1:2])
            cl = sb.tile([1, 1], F32)
            _tt(nc, cl[:], psS[0:1, 0:1], r2[:], A.mult)
            tmp2 = sb.tile([1, 1], F32)
            nc.vector.scalar_tensor_tensor(
                tmp2[:], iou[:], -1.0, cl[:], A.mult, A.add
            )
            lossv = sb.tile([1, 1], F32)
            nc.vector.tensor_scalar(lossv[:], tmp2[:], 1.0, None, A.add)
            nc.sync.dma_start(loss[:], lossv[:])

    for bb in nc.main_func.blocks:
        dead = [
            ins
            for ins in bb.instructions
            if type(ins).__name__ == "InstMemset" and "@const-" in ins.concise()
        ]
        for ins in dead:
            try:
                bb.instructions.remove(ins)
            except (ValueError, AttributeError):
                pass
    nc.compile()
    return nc


def _constants():
    ones1 = np.ones((1, 1), dtype=np.float32)
    ones32 = np.ones((B, 1), dtype=np.float32)
    emat = np.zeros((B, P), dtype=np.float32)
    for k in range(B):
        emat[k, k * SEG : (k + 1) * SEG] = 1.0
    ematt = np.ascontiguousarray(emat.T)
    sel4 = (np.arange(P)[:, None] % SEG == np.arange(SEG)[None, :]).astype(np.float32)
    resetv = np.zeros((1, P), dtype=np.float32)
    resetv[0, ::SEG] = NEGBIG
    id128 = np.eye(P, dtype=np.float32)
    iotag = (
        float(FREE) * np.arange(P, dtype=np.float32)[:, None]
        + np.arange(FREE, dtype=np.float32)[None, :]
        - GOFF
    ).astype(np.float32)
    # global row base in (idx - 65536) space relative to the (idx - 4096) space
    rb32 = np.repeat(
        (2048.0 * np.arange(B, dtype=np.float32) - (GOFF - OFF))[:, None], 2, axis=1
    ).astype(np.float32)
    rst1 = np.where(np.arange(P) % SEG == 0, NEGBIG, 0.0).reshape(P, 1)
    rst2 = np.where(np.arange(P) % SEG < 2, NEGBIG, 0.0).reshape(P, 1)
    pk1 = np.concatenate([sel4, ematt, rst1, rst2], axis=1).astype(np.float32)
    return {"pk1": pk1, "emat": emat, "ones32": ones32, "rb32": rb32, "iotag": iotag}


def build_in_maps(c_ids, gt_start_positions, gt_end_positions, start_logits, end_logits):
    consts = _constants()
    cidf = np.ascontiguousarray(np.asarray(c_ids), dtype=np.int16).reshape(P, FREE)
    slf = np.ascontiguousarray(np.asarray(start_logits), dtype=np.float32).reshape(P, FREE)
    elf = np.ascontiguousarray(np.asarray(end_logits), dtype=np.float32).reshape(P, FREE)
    gt32 = np.stack(
        [
            np.asarray(gt_start_positions).astype(np.float32) - np.float32(OFF),
            np.asarray(gt_end_positions).astype(np.float32) - np.float32(OFF),
        ],
        axis=1,
    ).astype(np.float32)
    pk2 = np.concatenate(
        [consts["emat"], consts["ones32"], gt32, consts["rb32"]], axis=1
    ).astype(np.float32)
    core_map = {
        "sl": slf, "el": elf, "cid": cidf,
        "pk1": consts["pk1"], "pk2": pk2, "iotag": consts["iotag"],
    }
    return [dict(core_map) for _ in range(NCORES)]


def kernel(c_ids, gt_start_positions, gt_end_positions, start_logits, end_logits):
    if "nc" not in _CACHE:
        _CACHE["nc"] = _build_program()
    nc = _CACHE["nc"]
    in_maps = build_in_maps(
        c_ids, gt_start_positions, gt_end_positions, start_logits, end_logits
    )
    res = run_bass_kernel_spmd(nc, in_maps, core_ids=list(range(NCORES)))
    return np.asarray(res.results[0]["loss"], dtype=np.float32).reshape(())


# revision 25
# speedup vs baseline: 1.1405x; 1.0008x over previous
"""Trainium2 Bass kernel for nn_DIoUAnswerSpanLoss.

Algorithm
---------
The reference builds a [B, L, L] score matrix score[b,i,j] = ls[b,i] + le[b,j]
(log-softmaxed logits), masks the lower triangle and pad positions, takes
argmax over (i, j), then computes a DIoU-style loss from the resulting integer
span positions (and the ground-truth spans).

Two exact reductions make this cheap:
  1. log_softmax subtracts a per-row constant, which never changes any
     argmax, so raw logits can be used directly for position finding.
  2. max_i<=j (msl[i] + el[j]) = cummax(msl)[j] + el[j], so the [L, L]
     matrix never needs materializing - a prefix max suffices.

Per row: msl = mask ? sl : -BIG; m = cummax(msl); col = mask ? m + el : -BIG;
j* = first argmax(col); v* = m[j*]; i* = first index with m == v*.

Distribution: the whole computation per core is ~15us of straight-line work,
while an 8-core AllReduce of even 16 bytes costs ~45us of latency on this
fabric. So instead of sharding the batch and reducing, every core receives
the full input and computes the full scalar loss independently - zero
communication - and the answer is read from core 0.

Layout: each of the 32 rows occupies 4 SBUF partitions x 512 elements
(p = 4*row + seg, global index g = 512*p + f - 65536, kept negative so a
zero from a non-matching lane never wins a reduce_min). The prefix max runs
as a segmented tensor_tensor_scan; segment boundaries are stitched by a
partition-shuffle Hillis-Steele pass (stream_shuffle shifts within each row's
4 partitions; a -1e30 additive mask kills shifts that crossed a row start).
Per-partition (max, first-argmax) come from the native MAX8/FIND_INDEX8 pair.
Cross-partition folds use fp32 matmuls with one-hot constants (exact: every
product is x*1.0 or x*0.0);
the per-row selection then runs in a [32 rows x 4 seg] domain where the row
reductions are plain per-partition reduces and every broadcast is a
per-partition tensor_scalar operand. First-occurrence argmax semantics
(matching jnp.argmax) fall out of reduce_min over (value==max)*(negative
global index). The global-index offset cancels in every DIoU term after a
single re-basing subtract, so ground-truth positions are passed pre-offset.
"""

import numpy as np

import concourse.bass as bass
import concourse.bacc as bacc
import concourse.mybir as mybir
from concourse import tile
from concourse.tile import add_dep_helper
from concourse.bass_utils import run_bass_kernel_spmd

B, L = 32, 2048
NCORES = 8
SEG = 4                    # segments (partitions) per row
FREE = L // SEG            # 512 elements per segment
P = B * SEG                # 128 partitions
NEGBIG = -1.0e30
GOFF = float(P * FREE)     # 65536: global index offset
OFF = 4096.0               # offset used for the DIoU position space

F32 = mybir.dt.float32
U16 = mybir.dt.uint16
A = mybir.AluOpType
AX = mybir.AxisListType.X

_CACHE = {}

MASK1 = [i - 1 if i % SEG else i for i in range(32)]
MASK2 = [i - 2 if i % SEG >= 2 else i for i in range(32)]


def _tt(nc, out, a, b, op):
    nc.vector.tensor_tensor(out, a, b, op)


def _build_program():
    nc = bacc.Bacc("TRN2", target_bir_lowering=False, debug=False, num_devices=NCORES)

    sl = nc.dram_tensor("sl", [P, FREE], F32, kind="ExternalInput")
    el = nc.dram_tensor("el", [P, FREE], F32, kind="ExternalInput")
    cid = nc.dram_tensor("cid", [P, FREE], mybir.dt.int16, kind="ExternalInput")
    # constants packed into three shape-compatible bundles (fewer DMAs)
    iotag = nc.dram_tensor("iotag", [P, FREE], F32, kind="ExternalInput")
    pk1 = nc.dram_tensor("pk1", [P, SEG + B + 2], F32, kind="ExternalInput")
    pk2 = nc.dram_tensor("pk2", [B, P + 5], F32, kind="ExternalInput")
    loss = nc.dram_tensor("loss", [1], F32, kind="ExternalOutput")

    with tile.TileContext(nc) as tc:
        with (
            tc.tile_pool(name="sb", bufs=1) as sb,
            tc.tile_pool(name="ps", bufs=4, space="PSUM") as ps,
        ):
            # ---- loads (critical ones first) ----
            sl_s = sb.tile([P, FREE], F32)
            cid_s = sb.tile([P, FREE], mybir.dt.int16)
            el_s = sb.tile([P, FREE], F32)
            iotag_t = sb.tile([P, FREE], F32)
            pk1_s = sb.tile([P, SEG + B + 2], F32)
            pk2_s = sb.tile([B, P + 5], F32)
            nc.sync.dma_start(sl_s[:], sl[:])
            nc.sync.dma_start(cid_s[:], cid[:])
            nc.sync.dma_start(pk1_s[:], pk1[:])
            nc.gpsimd.dma_start(pk2_s[:], pk2[:])
            nc.gpsimd.dma_start(el_s[:], el[:])
            nc.gpsimd.dma_start(iotag_t[:], iotag[:])
            iotag_s = iotag_t[:]
            sel4_s = pk1_s[:, 0:SEG]
            ematt_s = pk1_s[:, SEG : SEG + B]
            rst1_s = pk1_s[:, SEG + B : SEG + B + 1]
            rst2_s = pk1_s[:, SEG + B + 1 : SEG + B + 2]
            emat_s = pk2_s[:, 0:P]
            ones32_s = pk2_s[:, P : P + 1]
            gt32_s = pk2_s[:, P + 1 : P + 3]
            rb32_s = pk2_s[:, P + 3 : P + 5]

            # ---- masked start logits ----
            penalty = sb.tile([P, FREE], F32)
            nc.vector.tensor_scalar(
                penalty[:], cid_s[:], 0.0, NEGBIG, A.is_equal, A.mult
            )
            msl = sb.tile([P, FREE], F32)
            _tt(nc, msl[:], sl_s[:], penalty[:], A.add)

            # ---- segmented prefix max (within each 512-elem segment) ----
            mseg = sb.tile([P, FREE], F32)
            scan_i = nc.vector.tensor_tensor_scan(
                mseg[:], msl[:], msl[:], NEGBIG, A.max, A.max
            )

            # ---- stitch segments: exclusive cross-segment prefix max ----
            # partition-shuffle Hillis-Steele over the 4 segments of each row;
            # rst1/rst2 add -1e30 where a shift crossed a row boundary
            ends = mseg[:, FREE - 1 : FREE]
            sh1 = sb.tile([P, 1], F32)
            nc.vector.stream_shuffle(sh1[:], ends, MASK1)
            m1s = sb.tile([P, 1], F32)
            nc.vector.scalar_tensor_tensor(m1s[:], sh1[:], rst1_s, ends, A.add, A.max)
            sh2 = sb.tile([P, 1], F32)
            nc.vector.stream_shuffle(sh2[:], m1s[:], MASK2)
            m2s = sb.tile([P, 1], F32)
            nc.vector.scalar_tensor_tensor(m2s[:], sh2[:], rst2_s, m1s[:], A.add, A.max)
            sh3 = sb.tile([P, 1], F32)
            nc.vector.stream_shuffle(sh3[:], m2s[:], MASK1)
            exclf = sb.tile([P, 1], F32)
            nc.vector.tensor_scalar(exclf[:], sh3[:], rst1_s, None, A.add)

            # ---- column scores: col = max(mseg, excl) + (el + penalty) ----
            mel = sb.tile([P, FREE], F32)
            mel_i = nc.vector.tensor_tensor(mel[:], el_s[:], penalty[:], A.add)
            # keep the in-order DVE stream from head-of-line blocking on the
            # (later-arriving) el DMA: schedule mel after the scan
            add_dep_helper(mel_i.ins, scan_i.ins, sync=False,
                           reason="mel after scan: avoid DVE HOL block on el DMA")
            col = sb.tile([P, FREE], F32)
            nc.vector.scalar_tensor_tensor(
                col[:], mseg[:], exclf[:], mel[:], A.max, A.add
            )

            # ---- per-partition top-1 value + first index of col ----
            top8 = sb.tile([P, 8], F32)
            nc.vector.max(top8[:], col[:])
            pm = top8[:, 0:1]
            idx8 = sb.tile([P, 8], U16)
            nc.vector.max_index(idx8[:], top8[:], col[:])
            idxf = sb.tile([P, 1], F32)
            nc.vector.tensor_copy(idxf[:], idx8[:, 0:1])
            pjt = sb.tile([P, 1], F32)
            pj = pjt[:]
            nc.vector.tensor_scalar(pj, idxf[:], iotag_t[:, 0:1], None, A.add)

            # ---- per-partition m value at that index (exact gather) ----
            # gather mseg at the argmax position, then fold in the carried
            # prefix: m[pj] = max(mseg[pj], excl[partition])
            candv = sb.tile([P, FREE], F32)
            nc.vector.scalar_tensor_tensor(
                candv[:], iotag_s, pj, mseg[:], A.is_equal, A.mult
            )
            pvseg = sb.tile([P, 1], F32)
            nc.vector.reduce_sum(pvseg[:], candv[:], axis=AX)
            pvt = sb.tile([P, 1], F32)
            pv = pvt[:]
            nc.vector.tensor_scalar(pv, pvseg[:], exclf[:], None, A.max)

            # ---- fold [128,1] vectors into [32 rows x 4 seg] ----
            rhs12 = sb.tile([P, 3 * SEG], F32)
            nc.vector.tensor_scalar(rhs12[:, 0:SEG], sel4_s, pm, None, A.mult)
            nc.vector.tensor_scalar(
                rhs12[:, SEG : 2 * SEG], sel4_s, pj, None, A.mult
            )
            nc.vector.tensor_scalar(
                rhs12[:, 2 * SEG : 3 * SEG], sel4_s, pv, None, A.mult
            )
            psT = ps.tile([B, 3 * SEG], F32, tag="ps")
            nc.tensor.matmul(psT[:], ematt_s, rhs12[:])
            sT = sb.tile([B, 3 * SEG], F32)
            nc.vector.tensor_copy(sT[:], psT[:])
            pmr = sT[:, 0:SEG]
            pjr = sT[:, SEG : 2 * SEG]
            pvr = sT[:, 2 * SEG : 3 * SEG]

            # ---- per-row selection, all per-partition now ----
            m4 = sb.tile([B, 1], F32)
            nc.vector.reduce_max(m4[:], pmr, axis=AX)
            c1 = sb.tile([B, SEG], F32)
            nc.vector.tensor_scalar(c1[:], pmr, m4[:], None, A.is_equal)
            cj = sb.tile([B, SEG], F32)
            _tt(nc, cj[:], c1[:], pjr, A.mult)
            pos32 = sb.tile([B, 2], F32)
            j4 = pos32[:, 1:2]
            nc.vector.tensor_reduce(j4, cj[:], axis=AX, op=A.min)
            c2 = sb.tile([B, SEG], F32)
            nc.vector.tensor_scalar(c2[:], cj[:], j4, None, A.is_equal)
            cv = sb.tile([B, SEG], F32)
            _tt(nc, cv[:], c2[:], pvr, A.mult)
            v4 = sb.tile([B, 1], F32)
            nc.vector.reduce_sum(v4[:], cv[:], axis=AX)

            # ---- broadcast v* back per partition, find i* ----
            psVb = ps.tile([P, 1], F32, tag="ps")
            nc.tensor.matmul(psVb[:], emat_s, v4[:])
            candi = sb.tile([P, FREE], F32)
            nc.vector.scalar_tensor_tensor(
                candi[:], msl[:], psVb[:], iotag_s, A.is_equal, A.mult
            )
            pit = sb.tile([P, 1], F32)
            pi = pit[:]
            nc.vector.tensor_reduce(pi, candi[:], axis=AX, op=A.min)
            rhs4 = sb.tile([P, SEG], F32)
            nc.vector.tensor_scalar(rhs4[:], sel4_s, pi, None, A.mult)
            psI = ps.tile([B, SEG], F32, tag="ps")
            nc.tensor.matmul(psI[:], ematt_s, rhs4[:])
            i4 = pos32[:, 0:1]
            nc.vector.tensor_reduce(i4, psI[:], axis=AX, op=A.min)

            # ---- DIoU from positions, [32 x *] domain ----
            posn = sb.tile([B, 2], F32)     # i*-4096 || j*-4096 per row
            _tt(nc, posn[:], pos32[:], rb32_s, A.subtract)
            ct2 = sb.tile([B, 2], F32)      # te = ep-sp || tg = gep-gsp
            _tt(nc, ct2[:, 0:1], posn[:, 1:2], posn[:, 0:1], A.subtract)
            tgs_i = nc.vector.tensor_tensor(
                ct2[:, 1:2], gt32_s[:, 1:2], gt32_s[:, 0:1], A.subtract
            )
            add_dep_helper(tgs_i.ins, scan_i.ins, sync=False,
                           reason="t_gs after scan: avoid DVE HOL block on pk2 DMA")
            cdg = sb.tile([B, 2], F32)      # cd || gcd = (x+1)*0.5
            nc.vector.tensor_scalar(cdg[:], ct2[:], 1.0, 0.5, A.add, A.mult)
            mnx = sb.tile([B, 2], F32)      # min(sp,gsp) || min(ep,gep)
            _tt(nc, mnx[:], posn[:], gt32_s, A.min)
            mxx = sb.tile([B, 2], F32)      # max(sp,gsp) || max(ep,gep)
            _tt(nc, mxx[:], posn[:], gt32_s, A.max)
            dd = sb.tile([B, 2], F32)       # cd-gcd || max_end-min_start
            _tt(nc, dd[:, 0:1], cdg[:, 0:1], cdg[:, 1:2], A.subtract)
            _tt(nc, dd[:, 1:2], mxx[:, 1:2], mnx[:, 0:1], A.subtract)
            sq3 = sb.tile([B, 3], F32)      # d1^2 || d2^2 || te+tg
            _tt(nc, sq3[:, 0:2], dd[:], dd[:], A.mult)
            _tt(nc, sq3[:, 2:3], ct2[:, 0:1], ct2[:, 1:2], A.add)
            dI = sb.tile([1, 1], F32)       # I = min(ep,gep)[0] - max(sp,gsp)[0]
            _tt(nc, dI[:], mnx[0:1, 1:2], mxx[0:1, 0:1], A.subtract)
            psS = ps.tile([1, 3], F32, tag="ps")
            nc.tensor.matmul(psS[:], ones32_s, sq3[:])

            # loss = 1 - I/(s3 - 32*I) + s1/s2
            u = sb.tile([1, 1], F32)
            nc.vector.scalar_tensor_tensor(
                u[:], dI[:], -float(B), psS[0:1, 2:3], A.mult, A.add
            )
            ru = sb.tile([1, 1], F32)
            nc.vector.reciprocal(ru[:], u[:])
            iou = sb.tile([1, 1], F32)
            _tt(nc, iou[:], dI[:], ru[:], A.mult)
            r2 = sb.tile([1, 1], F32)
            nc.vector.reciprocal(r2[:], psS[0:1, 1:2])
            cl = sb.tile([1, 1], F32)
            _tt(nc, cl[:], psS[0:1, 0:1], r2[:], A.mult)
            tmp2 = sb.tile([1, 1], F32)
            nc.vector.scalar_tensor_tensor(
                tmp2[:], iou[:], -1.0, cl[:], A.mult, A.add
            )
            lossv = sb.tile([1, 1], F32)
            nc.vector.tensor_scalar(lossv[:], tmp2[:], 1.0, None, A.add)
            nc.sync.dma_start(loss[:], lossv[:])

    for bb in nc.main_func.blocks:
        dead = [
            ins
            for ins in bb.instructions
            if type(ins).__name__ == "InstMemset" and "@const-" in ins.concise()
        ]
        for ins in dead:
            try:
                bb.instructions.remove(ins)
            except (ValueError, AttributeError):
                pass
    nc.compile()
    return nc


def _constants():
    ones1 = np.ones((1, 1), dtype=np.float32)
    ones32 = np.ones((B, 1), dtype=np.float32)
    emat = np.zeros((B, P), dtype=np.float32)
    for k in range(B):
        emat[k, k * SEG : (k + 1) * SEG] = 1.0
    ematt = np.ascontiguousarray(emat.T)
    sel4 = (np.arange(P)[:, None] % SEG == np.arange(SEG)[None, :]).astype(np.float32)
    resetv = np.zeros((1, P), dtype=np.float32)
    resetv[0, ::SEG] = NEGBIG
    id128 = np.eye(P, dtype=np.float32)
    iotag = (
        float(FREE) * np.arange(P, dtype=np.float32)[:, None]
        + np.arange(FREE, dtype=np.float32)[None, :]
        - GOFF
    ).astype(np.float32)
    # global row base in (idx - 65536) space relative to the (idx - 4096) space
    rb32 = np.repeat(
        (2048.0 * np.arange(B, dtype=np.float32) - (GOFF - OFF))[:, None], 2, axis=1
    ).astype(np.float32)
    rst1 = np.where(np.arange(P) % SEG == 0, NEGBIG, 0.0).reshape(P, 1)
    rst2 = np.where(np.arange(P) % SEG < 2, NEGBIG, 0.0).reshape(P, 1)
    pk1 = np.concatenate([sel4, ematt, rst1, rst2], axis=1).astype(np.float32)
    return {"pk1": pk1, "emat": emat, "ones32": ones32, "rb32": rb32, "iotag": iotag}


def build_in_maps(c_ids, gt_start_positions, gt_end_positions, start_logits, end_logits):
    consts = _constants()
    cidf = np.ascontiguousarray(np.asarray(c_ids), dtype=np.int16).reshape(P, FREE)
    slf = np.ascontiguousarray(np.asarray(start_logits), dtype=np.float32).reshape(P, FREE)
    elf = np.ascontiguousarray(np.asarray(end_logits), dtype=np.float32).reshape(P, FREE)
    gt32 = np.stack(
        [
            np.asarray(gt_start_positions).astype(np.float32) - np.float32(OFF),
            np.asarray(gt_end_positions).astype(np.float32) - np.float32(OFF),
        ],
        axis=1,
    ).astype(np.float32)
    pk2 = np.concatenate(
        [consts["emat"], consts["ones32"], gt32, consts["rb32"]], axis=1
    ).astype(np.float32)
    core_map = {
        "sl": slf, "el": elf, "cid": cidf,
        "pk1": consts["pk1"], "pk2": pk2, "iotag": consts["iotag"],
    }
    return [dict(core_map) for _ in range(NCORES)]


def kernel(c_ids, gt_start_positions, gt_end_positions, start_logits, end_logits):
    if "nc" not in _CACHE:
        _CACHE["nc"] = _build_program()
    nc = _CACHE["nc"]
    in_maps = build_in_maps(
        c_ids, gt_start_positions, gt_end_positions, start_logits, end_logits
    )
    res = run_bass_kernel_spmd(nc, in_maps, core_ids=list(range(NCORES)))
    return np.asarray(res.results[0]["loss"], dtype=np.float32).reshape(())
